# revision 18
# baseline (speedup 1.0000x reference)
"""AttentionPool3D kernel for 8 Trainium2 NeuronCores — xbar edition.

Math (per batch b):
  qk      = queries @ Wk                      [Q, C]
  scores  = (qk @ xf) * C**-0.5               [Q, S]   (bk shifts cancel in softmax)
  e       = exp(scores)                        (scores ~ N(0,1): no max needed)
  l       = sum_s e                           [Q]
  t       = sum_s e[q,s] * xf[c,s]            [Q, C]
  attended= (t / l) @ Wv.T + bv               [Q, C]   (bv exact: sum attn = 1)
  out     = attended.flatten() @ Wo.T + bo    [OUT]

Sharding: 8 cores = 4 batches x 2 spatial halves (flash-style partial softmax,
combined on host along with the tiny projections, ~0.005% of total FLOPs).

Device kernel per core (all fp16 data, f32 accumulate), per group of 3 tiles
of T=2048 spatial positions (m = tile-in-group, using the PE's allowed PSUM
output partition bases {0, 32, 64}):
  - scores: sc[32m+q, 512-slice] = qk32_cb.T @ x_cb  (qk zero-padded to 32
    rows; x streams as matmul rhs so it never needs a PE weight load)
  - one exp per slice on ScalarE over all 96 partitions: psum f32 ->
    en96 fp16; accum_out gives the l-partials for free
  - xT via DMA-crossbar transpose (a ucode instruction that occupies the
    issuing HWDGE engine, so xT transposes are split sync/scalar):
    [128, 2, 2048] -> [128s, 32blk, 128c]
  - e-xbar per group: en96 [96, 2048] -> eT [128, 16, 96]
  - t-matmuls: t_ps[4, 256] += eT[:, sch, 32m+q].T @ [xT_cb0 | xT_cb1]
Host: t = out rows, l = sum of accum partials at rows 32m+q.
"""

import os
import sys

import numpy as np

for _p in ("/opt/trn_rl_repo", "/root/.axon_site/_ro/trn_rl_repo"):
    if os.path.isdir(_p) and _p not in sys.path:
        sys.path.append(_p)

import concourse.bass as bass
import concourse.tile as tile
from concourse import bacc, bass_utils, mybir
from concourse.bass import ts
from concourse.bass_utils import run_bass_kernel_spmd

F16 = mybir.dt.float16
F32 = mybir.dt.float32

B, C, D, H, W = 4, 256, 32, 48, 48
S = D * H * W            # 73728
Q, OUT = 4, 512
NCORES = 8
SHALF = S // 2           # 36864 per core
SCALE = C ** -0.5        # 1/16, folded into exp's affine
QP = 32                  # padded query rows (PE psum bases 0/32/64)
GT = 3                   # tiles per group

DEFAULT_CFG = dict(
    tile_t=2048,       # spatial tile size (one DMA / one xT xbar)
    slice_w=512,       # score-matmul psum slice width (1 psum bank)
    sc_bufs=4,
    bufs_x=6,
    bufs_xts=7,
    scalar_xT=(),      # xT xbars issued from scalar (rest sync);
                       # concurrent xbars on two engines corrupt data
    exbar_eng="sync",  # e-xbar engine
    dma="sync",        # x-stream DMA trigger ring: a plain descriptor push,
                       # interleaved with the xbar ucode via PF-deep prefetch
    prefetch=3,        # tiles of stream-DMA lookahead
)


def _build_program(reps=1, **over):
    cfg = dict(DEFAULT_CFG, **over)
    T = cfg["tile_t"]
    NT = SHALF // T
    NCH = T // 128           # chunks of 128 spatial per tile
    SW = cfg["slice_w"]
    NSL = T // SW            # score slices per tile
    NG = NT // GT            # groups
    NL = NG * NSL            # accum_out columns
    assert NT % GT == 0
    scalar_xT = set(cfg["scalar_xT"])

    nc = bacc.Bacc("TRN2", target_bir_lowering=False, debug=False,
                   num_devices=NCORES)
    xs = nc.dram_tensor("xs", [128, 2, SHALF], F16, kind="ExternalInput").ap()
    qkT = nc.dram_tensor("qkT", [128, 2, QP], F16, kind="ExternalInput").ap()
    out_tl = nc.dram_tensor("out_tl", [Q, C], F32, kind="ExternalOutput").ap()
    l_out = nc.dram_tensor("l_out", [GT * QP, NL], F32,
                           kind="ExternalOutput").ap()

    with tile.TileContext(nc) as tc:
        with (
            tc.tile_pool(name="consts", bufs=1) as consts,
            tc.tile_pool(name="xin", bufs=cfg["bufs_x"]) as xin_pool,
            tc.tile_pool(name="xts", bufs=cfg["bufs_xts"]) as xts_pool,
            tc.tile_pool(name="en", bufs=2) as en_pool,
            tc.tile_pool(name="et", bufs=2) as et_pool,
            tc.tile_pool(name="osb", bufs=1) as out_pool,
            tc.tile_pool(name="scps", bufs=cfg["sc_bufs"], space="PSUM") as sc_pool,
            tc.tile_pool(name="accps", bufs=1, space="PSUM") as acc_pool,
        ):
            qk_sb = consts.tile([128, 2, QP], F16)
            nc.sync.dma_start(qk_sb[:], qkT[:])
            l_sb = consts.tile([GT * QP, NL], F32)

            t_ps = acc_pool.tile([Q, C], F32)

            xt_tiles = {}
            PF = cfg["prefetch"]
            n_tiles = reps * NT

            def trigger_dma(itg):
                """Queue tile itg's stream DMA on the chosen HWDGE ring."""
                if itg >= n_tiles:
                    return
                xt = xin_pool.tile([128, 2, T], F16, name="xt", tag="xt")
                getattr(nc, cfg["dma"]).dma_start(
                    xt[:], xs[:, :, ts(itg % NT, T)])
                xt_tiles[itg] = xt

            def emit_front(gi, g, m):
                """xT-xbar + score matmuls for tile m of group gi."""
                itg = gi * GT + m
                trigger_dma(itg + PF)
                xt = xt_tiles.pop(itg)
                # xT via crossbar: out[p, blk, j] = xt2d[j, blk*128+p]
                xt_sb = xts_pool.tile([128, 2 * NCH, 128], F16, name="xts",
                                      tag="xts")
                eng = nc.scalar if (itg % NT) in scalar_xT else nc.sync
                eng.dma_start_transpose(
                    xt_sb[:], xt[:].rearrange("p a b -> p (a b)"))
                for sl in range(NSL):
                    sc = cur["sc"][sl]
                    for cb in range(2):
                        nc.tensor.matmul(
                            sc[QP * m:QP * (m + 1), :],
                            lhsT=qk_sb[:, cb, :],
                            rhs=xt[:, cb, ts(sl, SW)],
                            start=(cb == 0), stop=(cb == 1),
                        )
                cur["xts"].append(xt_sb)

            for _it in range(PF):
                trigger_dma(_it)

            def emit_back(prev):
                """e-xbar + t-matmuls for a finished group."""
                g = prev["g"]
                # eT: out[p, blk, j] = en96[j, blk*128+p]
                et = et_pool.tile([128, NCH, GT * QP], F16, name="et",
                                  tag="et")
                getattr(nc, cfg["exbar_eng"]).dma_start_transpose(
                    et[:], prev["en"][:])
                for m in range(GT):
                    gidx = prev["gidx"] * GT + m
                    xt_v = prev["xts"][m][:].rearrange(
                        "p (a b) c -> p b a c", a=2)
                    for sch in range(NCH):
                        # rhs: [128, 2, 128] = [xT_cb0 | xT_cb1]
                        nc.tensor.matmul(
                            t_ps[:],
                            lhsT=et[:, sch, QP * m:QP * m + Q],
                            rhs=xt_v[:, sch, :, :],
                            start=(gidx == 0 and sch == 0),
                            stop=(gidx == reps * NT - 1 and sch == NCH - 1),
                        )

            groups = [(rep, g) for rep in range(reps) for g in range(NG)]
            prev = None
            for gidx in range(len(groups) + 1):
                cur = None
                if gidx < len(groups):
                    rep, g = groups[gidx]
                    cur = dict(
                        g=g, gidx=gidx, xts=[],
                        en=en_pool.tile([GT * QP, T], F16, name="en96",
                                        tag="en96"),
                        sc=[sc_pool.tile([GT * QP, SW], F32, name="sc",
                                         tag="sc") for sl in range(NSL)],
                    )
                    emit_front(gidx, g, 0)
                if prev is not None:
                    emit_back(prev)
                if cur is not None:
                    for m in range(1, GT):
                        emit_front(gidx, g, m)
                    for sl in range(NSL):
                        li = g * NSL + sl
                        nc.scalar.activation(
                            cur["en"][:, ts(sl, SW)], cur["sc"][sl][:],
                            mybir.ActivationFunctionType.Exp, scale=SCALE,
                            accum_out=l_sb[:, li:li + 1])
                prev = cur

            out_sb = out_pool.tile([Q, C], F32)
            nc.vector.tensor_copy(out_sb[:], t_ps[:])
            nc.sync.dma_start(out_tl[:], out_sb[:])
            nc.sync.dma_start(l_out[:], l_sb[:])

    nc.compile()
    return nc


_NC_CACHE = {}


def _get_program(reps=1, **over):
    key = (reps, tuple(sorted(over.items())))
    if key not in _NC_CACHE:
        _NC_CACHE[key] = _build_program(reps, **over)
    return _NC_CACHE[key]


def _make_in_maps(x, queries, Wk):
    xf = np.ascontiguousarray(x.reshape(B, C, S))
    qk = (queries.astype(np.float64) @ Wk.astype(np.float64)).astype(np.float16)
    qk32 = np.zeros((QP, C), np.float16)
    qk32[0:Q] = qk
    # qkT[p, blk, j] = qk32[j, blk*128 + p]
    qkT = np.ascontiguousarray(qk32.T.reshape(2, 128, QP).transpose(1, 0, 2))
    in_maps = []
    for core in range(NCORES):
        b, h = divmod(core, 2)
        shard = xf[b, :, h * SHALF:(h + 1) * SHALF].astype(np.float16)
        # xs[p, blk, s] = xf[b, blk*128 + p, h*SHALF + s]
        xs = np.ascontiguousarray(shard.reshape(2, 128, SHALF).transpose(1, 0, 2))
        in_maps.append({"xs": xs, "qkT": qkT})
    return in_maps


def run_device(in_maps, trace=False, reps=1, **over):
    nc = _get_program(reps, **over)
    return run_bass_kernel_spmd(nc, in_maps, list(range(NCORES)),
                                trace=trace)


def _combine(results, Wv, bv, Wo, bo):
    Wv64 = Wv.astype(np.float64)
    Wo64 = Wo.astype(np.float64)
    out = np.empty((B, OUT), np.float32)
    for b in range(B):
        t = np.zeros((Q, C), np.float64)
        l = np.zeros(Q, np.float64)
        for r in (results[2 * b], results[2 * b + 1]):
            t += r["out_tl"].astype(np.float64)
            lo = r["l_out"].astype(np.float64)        # [GT*QP, NL]
            for q in range(Q):
                l[q] += sum(lo[QP * m + q, :].sum() for m in range(GT))
        attended = (t / l[:, None]) @ Wv64.T + bv.astype(np.float64)
        flat = attended.reshape(-1)          # [Q*C]
        out[b] = (flat @ Wo64.T + bo.astype(np.float64)).astype(np.float32)
    return out


def kernel(x, queries, Wk, bk, Wv, bv, Wo, bo):
    x = np.asarray(x, np.float32)
    queries = np.asarray(queries, np.float32)
    Wk = np.asarray(Wk, np.float32)
    Wv = np.asarray(Wv, np.float32)
    bv = np.asarray(bv, np.float32)
    Wo = np.asarray(Wo, np.float32)
    bo = np.asarray(bo, np.float32)
    # bk shifts every score of a (b, q) row by the same constant, which
    # cancels exactly in softmax; it does not affect the output.
    in_maps = _make_in_maps(x, queries, Wk)
    results = run_device(in_maps).results
    return _combine(results, Wv, bv, Wo, bo)


# revision 23
# speedup vs baseline: 2.0923x; 2.0923x over previous
"""AttentionPool3D kernel for 8 Trainium2 NeuronCores — fp16 fused-transpose.

Math (per batch b):
  qk      = queries @ Wk                      [Q, C]
  scores  = (qk @ xf) * C**-0.5               [Q, S]   (bk shifts cancel in softmax)
  e       = exp(scores)                        (scores ~ N(0,1): no max needed)
  l       = sum_s e                           [Q]
  t       = sum_s e[q,s] * xf[c,s]            [Q, C]
  attended= (t / l) @ Wv.T + bv               [Q, C]   (bv exact: sum attn = 1)
  out     = attended.flatten() @ Wo.T + bo    [OUT]

Sharding: 8 cores = 4 batches x 2 spatial halves (flash-style partial softmax,
combined on host along with the tiny projections, ~0.005% of total FLOPs).

Device kernel per core, all fp16 data with f32 PSUM accumulate.  x streams
from HBM once (fp16 halves the traffic of the f32 original).  Per 128-column
chunk of x and per c-block, one fused matmul computes transpose AND scores:
  [xT | scoresT] = x_chunk.T @ [I | qkT_cb]     (132-wide, 1 cyc/row in fp16)
PSUM slots are 1KB-aligned so no matmul dest straddles a 2KB bank.  Then:
  - exp on ScalarE from the PSUM score columns -> eT [128, NCH, 4] fp16
    (already s-on-partitions: no e transpose needed anywhere)
  - xT PSUM->SBUF copies alternate Vector / GpSimd (Scalar only does exp)
  - l via one tiny matmul per tile: l_ps[64, 2] += eT[128, 64].T @ ones
  - t-matmuls: t_ps[4, 256] += eT[:, sch, :].T @ [xT_cb0 | xT_cb1]
Host: t = out_tl, l[q] = sum_sch l_out[4*sch+q].
"""

import os
import sys

import numpy as np

for _p in ("/opt/trn_rl_repo", "/root/.axon_site/_ro/trn_rl_repo"):
    if os.path.isdir(_p) and _p not in sys.path:
        sys.path.append(_p)

import concourse.bass as bass
import concourse.tile as tile
from concourse import bacc, bass_utils, mybir
from concourse.bass import ts
from concourse.bass_utils import run_bass_kernel_spmd
from concourse.masks import make_identity

F16 = mybir.dt.float16
F32 = mybir.dt.float32

B, C, D, H, W = 4, 256, 32, 48, 48
S = D * H * W            # 73728
Q, OUT = 4, 512
NCORES = 8
SHALF = S // 2           # 36864 per core
SCALE = C ** -0.5        # 1/16, folded into exp's affine
FW = 132                 # fused matmul width: 128 xT cols + 4 score cols
SLOT = 256               # f32 slot per (chunk, cb) in the fused PSUM tile

DEFAULT_CFG = dict(
    tile_t=2048,       # spatial tile size (one DMA)
    xg=2,              # chunks per fused PSUM tile ([128, xg, 2, SLOT])
    fps_bufs=3,
    bufs_x=3,
    bufs_xts=4,
    dma="sync",        # x-stream DMA ring
)


def _build_program(reps=1, **over):
    cfg = dict(DEFAULT_CFG, **over)
    T = cfg["tile_t"]
    NT = SHALF // T
    NCH = T // 128           # chunks of 128 spatial per tile
    XG = cfg["xg"]
    NFG = NCH // XG          # fused psum tiles per tile
    n_tiles = reps * NT

    nc = bacc.Bacc("TRN2", target_bir_lowering=False, debug=False,
                   num_devices=NCORES)
    xs = nc.dram_tensor("xs", [128, 2, SHALF], F16, kind="ExternalInput").ap()
    qkT = nc.dram_tensor("qkT", [128, 2, Q], F16, kind="ExternalInput").ap()
    out_tl = nc.dram_tensor("out_tl", [Q, C], F32, kind="ExternalOutput").ap()
    l_out = nc.dram_tensor("l_out", [NCH * Q, 2], F32,
                           kind="ExternalOutput").ap()

    with tile.TileContext(nc) as tc:
        with (
            tc.tile_pool(name="consts", bufs=1) as consts,
            tc.tile_pool(name="xin", bufs=cfg["bufs_x"]) as xin_pool,
            tc.tile_pool(name="xts", bufs=cfg["bufs_xts"]) as xts_pool,
            tc.tile_pool(name="et", bufs=3) as et_pool,
            tc.tile_pool(name="osb", bufs=1) as out_pool,
            tc.tile_pool(name="fps", bufs=cfg["fps_bufs"],
                         space="PSUM") as fps_pool,
            tc.tile_pool(name="accps", bufs=1, space="PSUM") as acc_pool,
            tc.tile_pool(name="lps", bufs=1, space="PSUM") as l_pool,
        ):
            # constants: fused rhs [I(128) | qkT_cb(4)] fp16, ones for l
            ident_f = consts.tile([128, 128], F32)
            make_identity(nc, ident_f)
            qk_sb = consts.tile([128, 2, Q], F16)
            nc.sync.dma_start(qk_sb[:], qkT[:])
            frhs = consts.tile([128, 2, FW], F16)
            for cb in range(2):
                nc.vector.tensor_copy(frhs[:, cb, 0:128], ident_f[:])
                nc.vector.tensor_copy(frhs[:, cb, 128:FW], qk_sb[:, cb, :])
            ones_f = consts.tile([128, 2], F32)
            nc.gpsimd.memset(ones_f, 1.0)
            ones2 = consts.tile([128, 2], F16)
            nc.vector.tensor_copy(ones2[:], ones_f[:])

            t_ps = acc_pool.tile([Q, C], F32)
            l_ps = l_pool.tile([NCH * Q, 2], F32)

            for itg in range(n_tiles):
                it = itg % NT
                xt = xin_pool.tile([128, 2, T], F16, name="xt", tag="xt")
                getattr(nc, cfg["dma"]).dma_start(xt[:], xs[:, :, ts(it, T)])

                xt_sb = xts_pool.tile([128, 2 * NCH, 128], F16, name="xts",
                                      tag="xts")
                et = et_pool.tile([128, NCH, Q], F16, name="et", tag="et")

                for fg in range(NFG):
                    f_ps = fps_pool.tile([128, XG, 2, SLOT], F32, name="fps",
                                         tag="fps")
                    for j in range(XG):
                        sch = fg * XG + j
                        for cb in range(2):
                            nc.tensor.matmul(
                                f_ps[:, j, cb, 0:FW],
                                lhsT=xt[:, cb, ts(sch, 128)],
                                rhs=frhs[:, cb, :],
                                start=True, stop=True,
                            )
                    # per-c-block score partials s0, s1:
                    # e = exp((s0+s1)/16) = exp(s0/16) * exp(s1/16)
                    # (one PSUM-source exp on ScalarE, fp16 multiply on V)
                    e2 = et_pool.tile([128, XG, 2, Q], F16, name="e2",
                                      tag="e2", bufs=3)
                    nc.scalar.activation(
                        e2[:], f_ps[:, :, :, 128:FW],
                        mybir.ActivationFunctionType.Exp, scale=SCALE)
                    nc.vector.tensor_mul(et[:, ts(fg, XG), :],
                                         e2[:, :, 0, :], e2[:, :, 1, :])
                    # xT -> SBUF fp16, alternating Vector / Scalar.
                    # xt_sb blocks are cb-major ([cb*NCH + sch]) to match
                    # the t-matmul's rhs view.
                    src = f_ps[:, :, :, 0:128]
                    dst = xt_sb[:].rearrange(
                        "p (a b) c -> p b a c", a=2)[:, ts(fg, XG)]
                    if fg % 2 == 0:
                        nc.vector.tensor_copy(dst, src)
                    else:
                        nc.scalar.copy(dst, src)

                # l partials: l_ps[4*sch+q] += sum_s eT
                nc.tensor.matmul(
                    l_ps[:], lhsT=et[:].rearrange("p a q -> p (a q)"),
                    rhs=ones2[:],
                    start=(itg == 0), stop=(itg == n_tiles - 1),
                )

                xt_v = xt_sb[:].rearrange("p (a b) c -> p b a c", a=2)
                for sch in range(NCH):
                    # rhs: [128, 2, 128] = [xT_cb0 | xT_cb1]
                    nc.tensor.matmul(
                        t_ps[:],
                        lhsT=et[:, sch, :],
                        rhs=xt_v[:, sch, :, :],
                        start=(itg == 0 and sch == 0),
                        stop=(itg == n_tiles - 1 and sch == NCH - 1),
                    )

            out_sb = out_pool.tile([Q, C], F32)
            nc.vector.tensor_copy(out_sb[:], t_ps[:])
            nc.sync.dma_start(out_tl[:], out_sb[:])
            l_sb = out_pool.tile([NCH * Q, 2], F32)
            nc.vector.tensor_copy(l_sb[:], l_ps[:])
            nc.sync.dma_start(l_out[:], l_sb[:])

    nc.compile()
    return nc


_NC_CACHE = {}


def _get_program(reps=1, **over):
    key = (reps, tuple(sorted(over.items())))
    if key not in _NC_CACHE:
        _NC_CACHE[key] = _build_program(reps, **over)
    return _NC_CACHE[key]


def _make_in_maps(x, queries, Wk):
    xf = np.ascontiguousarray(x.reshape(B, C, S))
    qk = (queries.astype(np.float64) @ Wk.astype(np.float64)).astype(np.float16)
    # qkT[p, blk, j] = qk[j, blk*128 + p]
    qkT = np.ascontiguousarray(qk.T.reshape(2, 128, Q).transpose(1, 0, 2))
    in_maps = []
    for core in range(NCORES):
        b, h = divmod(core, 2)
        shard = xf[b, :, h * SHALF:(h + 1) * SHALF].astype(np.float16)
        # xs[p, blk, s] = xf[b, blk*128 + p, h*SHALF + s]
        xs = np.ascontiguousarray(shard.reshape(2, 128, SHALF).transpose(1, 0, 2))
        in_maps.append({"xs": xs, "qkT": qkT})
    return in_maps


def run_device(in_maps, trace=False, reps=1, **over):
    nc = _get_program(reps, **over)
    return run_bass_kernel_spmd(nc, in_maps, list(range(NCORES)),
                                trace=trace)


def _combine(results, Wv, bv, Wo, bo):
    Wv64 = Wv.astype(np.float64)
    Wo64 = Wo.astype(np.float64)
    out = np.empty((B, OUT), np.float32)
    for b in range(B):
        t = np.zeros((Q, C), np.float64)
        l = np.zeros(Q, np.float64)
        for r in (results[2 * b], results[2 * b + 1]):
            t += r["out_tl"].astype(np.float64)
            lo = r["l_out"].astype(np.float64)        # [NCH*Q, 2]
            for q in range(Q):
                l[q] += lo[q::Q, 0].sum()
        attended = (t / l[:, None]) @ Wv64.T + bv.astype(np.float64)
        flat = attended.reshape(-1)          # [Q*C]
        out[b] = (flat @ Wo64.T + bo.astype(np.float64)).astype(np.float32)
    return out


def kernel(x, queries, Wk, bk, Wv, bv, Wo, bo):
    x = np.asarray(x, np.float32)
    queries = np.asarray(queries, np.float32)
    Wk = np.asarray(Wk, np.float32)
    Wv = np.asarray(Wv, np.float32)
    bv = np.asarray(bv, np.float32)
    Wo = np.asarray(Wo, np.float32)
    bo = np.asarray(bo, np.float32)
    # bk shifts every score of a (b, q) row by the same constant, which
    # cancels exactly in softmax; it does not affect the output.
    in_maps = _make_in_maps(x, queries, Wk)
    results = run_device(in_maps).results
    return _combine(results, Wv, bv, Wo, bo)
